# revision 6
# baseline (speedup 1.0000x reference)
"""Trainium2 Bass kernel for a 2-layer LSTM decoder (5 steps, same input each step).

Reference computation (per step t = 0..4):
    g1 = emb @ Wih1.T + bih1 + h0 @ Whh1.T + bhh1          [B, 2048]
    h0, c0 = lstm_update(g1, c0)                            [B, 512]
    g2 = h0 @ Wih2.T + bih2 + h1 @ Whh2.T + bhh2            [B, 44]
    h1, c1 = lstm_update(g2, c1)                            [B, 11]
    out[t] = h1

Strategy: pure data parallel over 8 NeuronCores (batch 16384 -> 2048/core).
All state is kept TRANSPOSED in SBUF ([feature, batch]) so the recurrent
matmuls need no per-step transposes. Weights are pre-transposed on the host
and DMA'd straight into their SBUF layouts (no on-chip transpose phase).
Matmuls run in plain fp32 (4 cycles/row on the PE — the NEFF is ~1.5ms,
which is noise next to the host dispatch path) so the internal recurrence
matches the fp32 reference to rounding error; the recon output is stored
fp16 (h1 is in (-1,1); ~2e-4 absolute rounding) to halve the
device->host wire bytes. Biases are folded into the ScalarE activation
(per-partition bias operand).

Host side: the jitted 8-core executable and the device-resident input
buffers are cached across kernel() calls (identity/equality keyed), so
warm calls are a single pipelined execute+fetch round trip through the
axon relay (~110ms, dominated by relay latency; the per-step state
update stages new h in temporaries so recurrent matmuls always read the
previous step's h — updating in place would corrupt the recurrence).
"""

from concurrent.futures import ThreadPoolExecutor

import numpy as np

BATCH, EMB, HID, INP, STEP = 16384, 64, 512, 11, 5
NCORES = 8
BC = BATCH // NCORES  # per-core batch = 2048
NCH = 4               # batch chunks of 512 (PSUM bank free-dim)
CH = BC // NCH        # 512
G1 = 4 * HID          # 2048
G2 = 4 * INP          # 44

_cache = {}
LAST_EXEC_NS = None


def _build_program():
    from contextlib import ExitStack

    import concourse.mybir as mybir
    import concourse.tile as tile
    from concourse import bacc

    f32 = mybir.dt.float32
    f16 = mybir.dt.float16
    AF = mybir.ActivationFunctionType

    nc = bacc.Bacc("TRN2", target_bir_lowering=False, debug=False,
                   num_devices=NCORES)

    # ---- DRAM I/O (per-core shard of embT; weights replicated) ----
    # All weight tensors arrive pre-transposed / pre-padded from the host.
    embT_d = nc.dram_tensor("embT", [EMB, BC], f32, kind="ExternalInput").ap()
    wih1T_d = nc.dram_tensor("wih1T", [EMB, G1], f32,
                             kind="ExternalInput").ap()
    whh1T_d = nc.dram_tensor("whh1T", [HID, G1], f32,
                             kind="ExternalInput").ap()
    b1_d = nc.dram_tensor("b1", [128, 16], f32, kind="ExternalInput").ap()
    wih2T_d = nc.dram_tensor("wih2T", [HID, 128], f32,
                             kind="ExternalInput").ap()
    whh2T_d = nc.dram_tensor("whh2T", [INP, 128], f32,
                             kind="ExternalInput").ap()
    b2_d = nc.dram_tensor("b2", [128, 1], f32, kind="ExternalInput").ap()
    # fp16 output halves the device->host wire bytes (the dominant cost of
    # a warm call); h1 is in (-1, 1) so fp16 keeps ~5e-4 relative accuracy.
    recon_d = nc.dram_tensor("recon", [STEP, BC, INP], f16,
                             kind="ExternalOutput").ap()

    with tile.TileContext(nc) as tc, ExitStack() as top:
        pw = top.enter_context(tc.tile_pool(name="weights", bufs=1))
        pstate = top.enter_context(tc.tile_pool(name="state", bufs=1))
        ph1 = top.enter_context(tc.tile_pool(name="h1pool", bufs=2))

        embT = pw.tile([EMB, BC], f32, name="embT", tag="embT")
        wih1T = pw.tile([EMB, G1], f32, name="wih1T", tag="wih1T")
        whh1T = [pw.tile([128, G1], f32, name=f"whh1T{k}", tag=f"whh1T{k}")
                 for k in range(4)]
        # L2 gate dim padded to 32-partition strips: gate g lives at
        # partitions/cols 32g..32g+10 (engine APs need 32-aligned bases).
        wih2T = [pw.tile([128, 128], f32, name=f"wih2T{k}", tag=f"wih2T{k}")
                 for k in range(4)]
        whh2T = pw.tile([INP, 128], f32, name="whh2T", tag="whh2T")
        b1 = pw.tile([128, 16], f32, name="b1", tag="b1")
        b2 = pw.tile([128, 1], f32, name="b2", tag="b2")

        nc.sync.dma_start(embT[:], embT_d)
        nc.sync.dma_start(wih1T[:], wih1T_d)
        for k in range(4):
            nc.sync.dma_start(whh1T[k][:], whh1T_d[k * 128:(k + 1) * 128, :])
            nc.sync.dma_start(wih2T[k][:], wih2T_d[k * 128:(k + 1) * 128, :])
        nc.sync.dma_start(whh2T[:], whh2T_d)
        nc.sync.dma_start(b1[:], b1_d)
        nc.sync.dma_start(b2[:], b2_d)

        h0T = [pstate.tile([128, BC], f32, name=f"h0T{k}", tag=f"h0T{k}")
               for k in range(4)]
        c0T = [pstate.tile([128, BC], f32, name=f"c0T{k}", tag=f"c0T{k}")
               for k in range(4)]
        c1 = pstate.tile([INP, BC], f32, name="c1", tag="c1")

        with ExitStack() as pmain:
            psum1 = pmain.enter_context(
                tc.tile_pool(name="psum1", bufs=6, space="PSUM"))
            psum2 = pmain.enter_context(
                tc.tile_pool(name="psum2", bufs=2, space="PSUM"))
            pg = pmain.enter_context(tc.tile_pool(name="gates", bufs=2))
            ptmp = pmain.enter_context(tc.tile_pool(name="tmp", bufs=2))
            pg2 = pmain.enter_context(tc.tile_pool(name="g2", bufs=2))
            phn = pmain.enter_context(tc.tile_pool(name="hnew", bufs=2))

            GATE_FN = [AF.Sigmoid, AF.Sigmoid, AF.Tanh, AF.Sigmoid]
            h1_prev = None

            for t in range(STEP):
                # ======== layer 1, n-major over batch chunks ========
                for n in range(NCH):
                    ns = slice(n * CH, (n + 1) * CH)
                    # New h for this batch chunk is staged in hn[] and only
                    # copied into h0T after ALL four k-blocks have run their
                    # recurrent matmuls — those must see the PREVIOUS step's
                    # h0 (writing h0T[k] in place inside the k loop would
                    # feed already-updated state to later k-blocks).
                    hn = []
                    for k in range(4):
                        gt = []  # sigmoid(i), sigmoid(f), tanh(g), sigmoid(o)
                        for g in range(4):
                            m = g * 4 + k
                            ps = psum1.tile([128, CH], f32, name="ps", tag="ps")
                            nc.tensor.matmul(
                                ps[:],
                                wih1T[:, m * 128:(m + 1) * 128],
                                embT[:, ns],
                                start=True, stop=(t == 0))
                            if t > 0:
                                for kk in range(4):
                                    nc.tensor.matmul(
                                        ps[:],
                                        whh1T[kk][:, m * 128:(m + 1) * 128],
                                        h0T[kk][:, ns],
                                        start=False, stop=(kk == 3))
                            gact = pg.tile([128, CH], f32, name=f"g{g}",
                                           tag=f"g{g}")
                            nc.scalar.activation(gact[:], ps[:], GATE_FN[g],
                                                 bias=b1[:, m:m + 1])
                            gt.append(gact)

                        # c = sig(f)*c + sig(i)*tanh(g); h = sig(o)*tanh(c)
                        if t > 0:
                            t1 = ptmp.tile([128, CH], f32, name="t1", tag="t1")
                            t2 = ptmp.tile([128, CH], f32, name="t2", tag="t2")
                            nc.vector.tensor_mul(t1[:], gt[0][:], gt[2][:])
                            nc.vector.tensor_mul(t2[:], c0T[k][:, ns], gt[1][:])
                            nc.vector.tensor_add(c0T[k][:, ns], t1[:], t2[:])
                        else:
                            nc.vector.tensor_mul(c0T[k][:, ns], gt[0][:],
                                                 gt[2][:])
                        th = ptmp.tile([128, CH], f32, name="th", tag="th")
                        nc.scalar.activation(th[:], c0T[k][:, ns], AF.Tanh)
                        hk = phn.tile([128, CH], f32, name=f"hn{k}",
                                      tag=f"hn{k}")
                        nc.vector.tensor_mul(hk[:], gt[3][:], th[:])
                        hn.append(hk)
                    for k in range(4):
                        nc.vector.tensor_copy(h0T[k][:, ns], hn[k][:])

                # ======== layer 2 ========
                h1_new = ph1.tile([INP, BC], f32, name="h1", tag="h1")
                h1h = ph1.tile([INP, BC], f16, name="h1h", tag="h1h")
                for n in range(NCH):
                    ns = slice(n * CH, (n + 1) * CH)
                    ps2 = psum2.tile([128, CH], f32, name="ps2", tag="ps2")
                    for kk in range(4):
                        nc.tensor.matmul(
                            ps2[:], wih2T[kk][:],
                            h0T[kk][:, ns],
                            start=(kk == 0),
                            stop=(kk == 3 and t == 0))
                    if t > 0:
                        nc.tensor.matmul(
                            ps2[:], whh2T[:],
                            h1_prev[0:INP, ns],
                            start=False, stop=True)

                    g2t = []
                    for g in range(4):
                        gs = slice(32 * g, 32 * g + INP)
                        ga = pg2.tile([INP, CH], f32, name=f"g2x{g}",
                                      tag=f"g2x{g}")
                        nc.scalar.activation(ga[:], ps2[gs, :],
                                             GATE_FN[g], bias=b2[gs, 0:1])
                        g2t.append(ga)
                    i2, f2, g2_, o2 = (x[:] for x in g2t)
                    if t > 0:
                        t1 = ptmp.tile([128, CH], f32, name="t1", tag="t1")
                        t2 = ptmp.tile([128, CH], f32, name="t2", tag="t2")
                        nc.vector.tensor_mul(t1[0:INP, :], i2, g2_)
                        nc.vector.tensor_mul(t2[0:INP, :], c1[:, ns], f2)
                        nc.vector.tensor_add(c1[:, ns], t1[0:INP, :],
                                             t2[0:INP, :])
                    else:
                        nc.vector.tensor_mul(c1[:, ns], i2, g2_)
                    th = ptmp.tile([128, CH], f32, name="th", tag="th")
                    nc.scalar.activation(th[0:INP, :], c1[:, ns], AF.Tanh)
                    nc.vector.tensor_mul(h1_new[0:INP, ns], o2, th[0:INP, :])

                # store h1 for step t: recon[t][b, i] <- h1_new[i, b]
                nc.vector.tensor_copy(h1h[:], h1_new[:])
                nc.sync.dma_start(recon_d[t].rearrange("b i -> i b"),
                                  h1h[:])
                h1_prev = h1_new

    nc.compile()
    return nc


def _get_executor():
    if "exec" in _cache:
        return _cache["exec"]

    import jax
    from jax.experimental.shard_map import shard_map
    from jax.sharding import Mesh, PartitionSpec

    import concourse.mybir as mybir
    from concourse.bass2jax import (_bass_exec_p, install_neuronx_cc_hook,
                                    partition_id_tensor)

    nc = _build_program()
    install_neuronx_cc_hook()

    partition_name = (nc.partition_id_tensor.name
                      if nc.partition_id_tensor else None)
    in_names = []
    out_names = []
    out_avals = []
    for alloc in nc.m.functions[0].allocations:
        if not isinstance(alloc, mybir.MemoryLocationSet):
            continue
        name = alloc.memorylocations[0].name
        if alloc.kind == "ExternalInput":
            if name != partition_name:
                in_names.append(name)
        elif alloc.kind == "ExternalOutput":
            out_names.append(name)
            out_avals.append(jax.core.ShapedArray(
                tuple(alloc.tensor_shape), mybir.dt.np(alloc.dtype)))
    n_params = len(in_names)
    # Our kernel writes every element of every output, so no pre-zeroed
    # output operands are needed: the NEFF binds its outputs to the
    # custom-call result buffers directly (output{i} rename in
    # neuronx_cc_hook), and the zero/donation dance in run_bass_via_pjrt
    # exists only for kernels that leave output elements unwritten.
    all_in_names = tuple(in_names)
    if partition_name is not None:
        all_in_names = all_in_names + (partition_name,)

    def _body(*args):
        operands = list(args)
        if partition_name is not None:
            operands.append(partition_id_tensor())
        outs = _bass_exec_p.bind(
            *operands,
            out_avals=tuple(out_avals),
            in_names=all_in_names,
            out_names=tuple(out_names),
            lowering_input_output_aliases=(),
            sim_require_finite=True,
            sim_require_nnan=True,
            nc=nc,
        )
        return tuple(outs)

    devices = jax.devices()[:NCORES]
    mesh = Mesh(np.asarray(devices), ("core",))
    in_specs = (PartitionSpec("core"),) * n_params
    out_specs = (PartitionSpec("core"),) * len(out_names)
    fn = jax.jit(
        shard_map(_body, mesh=mesh, in_specs=in_specs, out_specs=out_specs,
                  check_rep=False),
        keep_unused=True)

    from jax.sharding import NamedSharding
    sh = NamedSharding(mesh, PartitionSpec("core"))

    _cache["exec"] = (fn, tuple(in_names), tuple(out_names), out_avals, mesh,
                      sh)
    return _cache["exec"]


def _same(src_tuple, cached_tuple):
    """Cheap input-change detection: object identity, then exact compare."""
    if cached_tuple is None or len(src_tuple) != len(cached_tuple):
        return False
    if all(a is b for a, b in zip(src_tuple, cached_tuple)):
        return True
    return all(np.array_equal(a, b) for a, b in zip(src_tuple, cached_tuple))


def kernel(**inputs) -> np.ndarray:
    import os
    import time
    import jax

    prof = bool(int(os.environ.get("KERNEL_PROF", "0")))
    tA = time.perf_counter()
    fn, in_names, out_names, out_avals, mesh, sh = _get_executor()

    f = lambda x: np.ascontiguousarray(np.asarray(x), dtype=np.float32)
    emb = f(inputs["emb_inp"])
    Wih1, Whh1 = f(inputs["Wih1"]), f(inputs["Whh1"])
    bih1, bhh1 = f(inputs["bih1"]), f(inputs["bhh1"])
    Wih2, Whh2 = f(inputs["Wih2"]), f(inputs["Whh2"])
    bih2, bhh2 = f(inputs["bih2"]), f(inputs["bhh2"])

    src_for = {
        "embT": (emb,), "wih1T": (Wih1,), "whh1T": (Whh1,),
        "b1": (bih1, bhh1), "wih2T": (Wih2,), "whh2T": (Whh2,),
        "b2": (bih2, bhh2),
    }

    def build(name):
        """Global [NCORES*rows, cols] host array for NEFF input `name`."""
        if name == "embT":
            # per-core shard c: emb[c*BC:(c+1)*BC].T  -> [EMB, BC]
            return np.ascontiguousarray(
                emb.reshape(NCORES, BC, EMB).transpose(0, 2, 1)
            ).reshape(NCORES * EMB, BC)
        if name == "wih1T":
            base = np.ascontiguousarray(Wih1.T)            # [EMB, G1]
        elif name == "whh1T":
            base = np.ascontiguousarray(Whh1.T)            # [HID, G1]
        elif name == "b1":
            base = np.ascontiguousarray(
                (bih1 + bhh1).reshape(16, 128).T)          # [128, 16]
        elif name == "wih2T":
            base = np.zeros((HID, 128), np.float32)
            for g in range(4):
                base[:, 32 * g:32 * g + INP] = Wih2[g * INP:(g + 1) * INP].T
        elif name == "whh2T":
            base = np.zeros((INP, 128), np.float32)
            for g in range(4):
                base[:, 32 * g:32 * g + INP] = Whh2[g * INP:(g + 1) * INP].T
        elif name == "b2":
            base = np.zeros((128, 1), np.float32)
            bb = bih2 + bhh2
            for g in range(4):
                base[32 * g:32 * g + INP, 0] = bb[g * INP:(g + 1) * INP]
        else:
            raise KeyError(name)
        return np.ascontiguousarray(
            np.broadcast_to(base, (NCORES,) + base.shape)
        ).reshape(NCORES * base.shape[0], *base.shape[1:])

    dev = _cache.setdefault("dev", {})
    args = []
    tB = time.perf_counter()
    for name in in_names:
        srcs = src_for[name]
        ent = dev.get(name)
        if ent is None or not _same(srcs, ent[0]):
            dev[name] = (srcs, jax.device_put(build(name), sh))
        args.append(dev[name][1])
    tC = time.perf_counter()

    outs = fn(*args)
    tD = time.perf_counter()
    # Fetch shards concurrently and cast each straight into the output
    # buffer as it lands, overlapping the fp16->fp32 interleave with the
    # remaining wire time. out[t, c*BC:(c+1)*BC] = shard_c[t].
    out = np.empty((STEP, BATCH, INP), np.float32)
    ov = out.reshape(STEP, NCORES, BC, INP)
    try:
        shards = list(outs[0].addressable_shards)
        if len(shards) != NCORES:
            raise ValueError(f"expected {NCORES} shards, got {len(shards)}")

        def _fetch(s):
            start = s.index[0].start or 0
            ov[:, start // STEP] = np.asarray(s.data)

        pool = _cache.get("pool")
        if pool is None:
            pool = _cache["pool"] = ThreadPoolExecutor(NCORES)
        list(pool.map(_fetch, shards))
    except Exception:
        rec = np.asarray(outs[0])  # [NCORES*STEP, BC, INP] fp16
        np.copyto(ov, rec.reshape(NCORES, STEP, BC, INP).transpose(1, 0, 2, 3))
    tE = time.perf_counter()
    if prof:
        tF = time.perf_counter()
        print(f"[kernel] prep={1e3 * (tB - tA):.1f}ms "
              f"cache/upload={1e3 * (tC - tB):.1f}ms "
              f"dispatch={1e3 * (tD - tC):.1f}ms "
              f"fetch={1e3 * (tE - tD):.1f}ms "
              f"reshape={1e3 * (tF - tE):.1f}ms")
    return out
